# revision 41
# baseline (speedup 1.0000x reference)
"""TRN2 Bass kernel: relu + per-row top-32 masking for x [4096, 32768] f32.

kernel(x) -> (relu(x), topk_masked) matching:
    y = relu(x); vals, idx = top_k(y, 32); xz = zeros.at[rows, idx].set(vals)

Sharding: pure data parallel over rows, 8 NeuronCores x [512, 32768].

Per-core algorithm (exact for any input with >=32 positive entries per row):
  stream x in column sub-tiles: relu on ScalarE -> write y; chunk maxes
  (256 chunks of 128) on VectorE. Top-32 chunks via 4 rounds of DVE
  max8 + match_replace on a copy (selection mask = work != orig, which
  reproduces stable lower-index tie-breaking exactly); compact selected
  chunk ids by max8-extracting (256-c)*sel; indirect-DMA gather those 32
  chunks/row from DRAM; same max8 trick picks the top-32 elements; the
  masked chunks (value * keep) are indirect-DMA scattered back into the
  pre-zeroed xz output, so only ~1/8 of xz is ever written.
"""

import os
import sys

if "/opt/trn_rl_repo" not in sys.path:
    sys.path.insert(0, "/opt/trn_rl_repo")

import numpy as np

import concourse.bass as bass
import concourse.mybir as mybir
from concourse import bacc
from concourse.bass_utils import run_bass_kernel_spmd
from concourse.tile import TileContext

F32 = mybir.dt.float32
I32 = mybir.dt.int32
I16 = mybir.dt.int16

N_ROWS = 4096
N_COLS = 32768
N_CORES = 8
L = 128          # chunk length
K = 32           # top-k
P = 128          # rows per block (partitions)

LAST_EXEC_TIME_NS = None
LAST_TRACE_DIR = None
_CACHED_NC = None


def _set_prio(handles, delta):
    """Shift bass_priority of emitted instructions (smaller = scheduled first
    among co-pending instructions in Tile's priority heap)."""
    for h in handles:
        ins = getattr(h, "ins", h)
        if ins.bass_priority is not None:
            ins.bass_priority += delta


def _build(R: int, D: int, sub: int = 4096, g_bufs: int = 2, x_bufs: int = 4,
           m_bufs: int = 3, s_bufs: int = 4, prio: bool = True, cl: int = L,
           mode: str = "ind", scratch: int = 16384):
    assert mode in ("ind", "dg")
    if mode == "dg":
        assert cl == 128, "custom dma_gather path needs 512B chunks + int16 idx"
    C = D // cl
    n_blocks = R // P
    n_sub = D // sub
    sub_chunks = sub // cl

    nc = bacc.Bacc("TRN2", target_bir_lowering=False, debug=False,
                   dynamic_dma_scratch_size=scratch)
    x = nc.declare_dram_parameter("x", [R, D], F32, isOutput=False)
    y = nc.declare_dram_parameter("y", [R, D], F32, isOutput=True)
    xz = nc.declare_dram_parameter("xz", [R, D], F32, isOutput=True)

    x_chunks = x[:].rearrange("r (c l) -> (r c) l", l=cl)
    xz_chunks = xz[:].rearrange("r (c l) -> (r c) l", l=cl)

    with TileContext(nc) as tc:
        with (
            tc.tile_pool(name="consts", bufs=1) as const_pool,
            tc.tile_pool(name="xstream", bufs=x_bufs) as x_pool,
            tc.tile_pool(name="mstage", bufs=m_bufs) as m_pool,
            tc.tile_pool(name="gstage", bufs=g_bufs) as g_pool,
            tc.tile_pool(name="small", bufs=s_bufs) as s_pool,
        ):
            # ids_iota[p, c] = C - c  (max8 extraction then yields ascending chunk id)
            ids_iota_i = const_pool.tile([P, C], I32, tag="ids_iota_i")
            nc.gpsimd.iota(ids_iota_i[:], pattern=[[-1, C]], base=C, channel_multiplier=0)
            ids_iota = const_pool.tile([P, C], F32, tag="ids_iota")
            nc.vector.tensor_copy(ids_iota[:], ids_iota_i[:])
            # rowbase[p, 0] = p*C + C  (global chunk = rowbase - e + block_base)
            rowbase_i = const_pool.tile([P, 1], I32, tag="rowbase_i")
            nc.gpsimd.iota(rowbase_i[:], pattern=[[0, 1]], base=C, channel_multiplier=C)
            rowbase = const_pool.tile([P, 1], F32, tag="rowbase")
            nc.vector.tensor_copy(rowbase[:], rowbase_i[:])

            # Pipelined emission: [stream(b), M(b), gathers(b)] then
            # [gstage(b-1), scatters(b-1)]. Engines execute their scheduled
            # streams in order, so this keeps the gather critical path (reduces
            # -> M-rounds -> offsets -> gather DGE) ahead of the previous
            # block's G-stage on both DVE and Pool.
            pending = None  # (b, offs, G) awaiting gstage + scatters
            # All scatters write the same full-tensor xz AP (the indirect side
            # must have offset 0), so Tile chains them with WAW completion
            # semaphores. The actual chunk destinations are provably disjoint
            # (distinct chunks per row, distinct rows per block), so strip
            # scatter->scatter deps.
            scatter_names = set()

            def emit_gstage_and_scatter(state):
                sb, s_sel, G = state
                Gf = G[:].rearrange("p k l -> p (k l)")
                # top-32 elements of G: zap to 0, then masked = Gf - Gw
                # (kept values are > 0 whenever every row has >= 32 positives,
                #  so zapped zeros are never re-extracted)
                Gw = g_pool.tile([P, K * cl], F32, tag="Gw")
                gx8 = s_pool.tile([P, 8], F32, tag="gx8")
                gsrc = Gf
                for _ in range(K // 8):
                    nc.vector.max(gx8[:], gsrc)
                    nc.vector.match_replace(out=Gw[:], in_to_replace=gx8[:],
                                            in_values=gsrc, imm_value=0.0)
                    gsrc = Gw[:]
                nc.vector.tensor_tensor(out=Gw[:], in0=Gf, in1=Gw[:],
                                        op=mybir.AluOpType.subtract)
                Gw3 = Gw[:].rearrange("p (k l) -> p k l", l=cl)
                if mode == "dg":
                    # scatter-add the masked chunks onto the pre-zeroed xz
                    # (one custom SWDGE instruction; add-to-zero == write)
                    ins = nc.gpsimd.dma_scatter_add(
                        out_ap=xz_chunks[sb * P * C:(sb + 1) * P * C, :],
                        in_ap=Gw3,
                        idxs_ap=s_sel[:],
                        num_idxs=P * K,
                        num_idxs_reg=P * K,
                        elem_size=cl,
                        single_packet=False,
                    )
                    new_ins = [ins]
                else:
                    new_ins = []
                    for k in range(K):
                        new_ins.append(nc.gpsimd.indirect_dma_start(
                            out=xz_chunks,
                            out_offset=bass.IndirectOffsetOnAxis(ap=s_sel[:, k:k + 1], axis=0),
                            in_=Gw3[:, k, :],
                            in_offset=None,
                        ))
                for ins in new_ins:
                    ins = getattr(ins, "ins", ins)
                    for dep in list(ins.sync_dependency_names()):
                        if dep in scatter_names:
                            ins.try_remove_dependency(dep)
                    scatter_names.add(ins.name)

            for b in range(n_blocks):
                r0 = b * P
                M = m_pool.tile([P, C], F32, tag="M")
                stream_h = []
                w_h = []
                for s in range(n_sub):
                    c0 = s * sub
                    xt = x_pool.tile([P, sub], F32, tag="xt")
                    stream_h.append(nc.sync.dma_start(out=xt[:], in_=x[r0:r0 + P, c0:c0 + sub]))
                    stream_h.append(nc.scalar.activation(xt[:], xt[:], mybir.ActivationFunctionType.Relu))
                    w_h.append(nc.sync.dma_start(out=y[r0:r0 + P, c0:c0 + sub], in_=xt[:]))
                    stream_h.append(nc.vector.tensor_reduce(
                        out=M[:, s * sub_chunks:(s + 1) * sub_chunks],
                        in_=xt[:].rearrange("p (c l) -> p c l", l=cl),
                        axis=mybir.AxisListType.X,
                        op=mybir.AluOpType.max,
                    ))

                # top-32 chunks (first round reads M directly, rest in-place on Mw)
                m_h = []
                Mw = m_pool.tile([P, C], F32, tag="Mw")
                mx8 = s_pool.tile([P, 8], F32, tag="mx8")
                src = M
                for _ in range(K // 8):
                    m_h.append(nc.vector.max(mx8[:], src[:]))
                    m_h.append(nc.vector.match_replace(out=Mw[:], in_to_replace=mx8[:],
                                                       in_values=src[:], imm_value=-1.0))
                    src = Mw
                selM = m_pool.tile([P, C], F32, tag="selM")
                m_h.append(nc.vector.tensor_tensor(out=selM[:], in0=Mw[:], in1=M[:],
                                                   op=mybir.AluOpType.not_equal))
                ids = m_pool.tile([P, C], F32, tag="ids")
                m_h.append(nc.vector.tensor_tensor(out=ids[:], in0=selM[:], in1=ids_iota[:],
                                                   op=mybir.AluOpType.mult))
                idsel = s_pool.tile([P, K], F32, tag="idsel")
                offs_f = s_pool.tile([P, K], F32, tag="offs_f")
                g_h = []
                G = g_pool.tile([P, K, cl], F32, tag="G")

                if mode == "ind":
                    # Interleave id-extraction rounds with per-group offset
                    # math and gather issue, so gather DGE overlaps the
                    # remaining DVE id rounds.
                    sel = s_pool.tile([P, K], I32, tag="offs")
                    for r in range(K // 8):
                        sl = slice(r * 8, (r + 1) * 8)
                        m_h.append(nc.vector.max(idsel[:, sl], ids[:]))
                        if r < K // 8 - 1:
                            m_h.append(nc.vector.match_replace(
                                out=ids[:], in_to_replace=idsel[:, sl],
                                in_values=ids[:], imm_value=0.0))
                        m_h.append(nc.vector.tensor_scalar(
                            offs_f[:, sl], idsel[:, sl], -1.0, None,
                            op0=mybir.AluOpType.mult))
                        m_h.append(nc.vector.tensor_scalar(
                            offs_f[:, sl], offs_f[:, sl], rowbase[:, :1],
                            float(b * P * C),
                            op0=mybir.AluOpType.add, op1=mybir.AluOpType.add))
                        m_h.append(nc.vector.tensor_copy(sel[:, sl], offs_f[:, sl]))
                        for k in range(r * 8, (r + 1) * 8):
                            g_h.append(nc.gpsimd.indirect_dma_start(
                                out=G[:, k, :], out_offset=None,
                                in_=x_chunks,
                                in_offset=bass.IndirectOffsetOnAxis(ap=sel[:, k:k + 1], axis=0),
                            ))
                    if prio:
                        _set_prio(stream_h, -3_000_000)
                        _set_prio(w_h, -3_000_000)
                        _set_prio(m_h, -2_000_000)
                        _set_prio(g_h, -1_000_000)
                    if pending is not None:
                        emit_gstage_and_scatter(pending)
                    pending = (b, sel, G)
                    continue

                for r in range(K // 8):
                    m_h.append(nc.vector.max(idsel[:, r * 8:(r + 1) * 8], ids[:]))
                    m_h.append(nc.vector.match_replace(out=ids[:], in_to_replace=idsel[:, r * 8:(r + 1) * 8],
                                                       in_values=ids[:], imm_value=0.0))

                # offsets: block-relative chunk index = rowbase - e
                # = p*C + (C - e) <= P*C - 1, exact in f32.
                m_h.append(nc.vector.tensor_scalar(offs_f[:], idsel[:], -1.0, None,
                                                   op0=mybir.AluOpType.mult))
                if mode == "dg":
                    m_h.append(nc.vector.tensor_scalar(offs_f[:], offs_f[:], rowbase[:, :1],
                                                       None, op0=mybir.AluOpType.add))
                    offs16 = s_pool.tile([P, K], I16, tag="offs16")
                    m_h.append(nc.vector.tensor_copy(offs16[:], offs_f[:]))
                    # dma_gather idx layout: int16 entries wrapped in 16
                    # partitions (entry n at [n % 16, n // 16]), replicated to
                    # all 8 Q7 core groups; gathered segment i lands at
                    # (partition i % 128, slot i // 128). We want G[p, j] =
                    # chunk offs16[p, j], i.e. idx[j*128 + p] = offs16[p, j],
                    # i.e. IDX16[q, j*8 + g] = offs16[g*16 + q, j].
                    sel = s_pool.tile([P, K * 8], I16, tag="idx16")
                    for g in range(8):
                        m_h.append(nc.sync.dma_start(
                            out=sel[0:16, g::8],
                            in_=offs16[g * 16:(g + 1) * 16, :],
                        ))
                    for grp in range(1, 8):
                        m_h.append(nc.sync.dma_start(
                            out=sel[grp * 16:(grp + 1) * 16, :],
                            in_=sel[0:16, :],
                        ))
                    g_h.append(nc.gpsimd.dma_gather(
                        out_ap=G[:],
                        in_ap=x_chunks[b * P * C:(b + 1) * P * C, :],
                        idxs_ap=sel[:],
                        num_idxs=P * K,
                        num_idxs_reg=P * K,
                        elem_size=cl,
                        single_packet=False,
                    ))
                else:
                    # global chunk index = rowbase - e + b*P*C; one indirect
                    # DMA per chunk ordinal (walrus supports one dynamic
                    # offset per partition, 2D SBUF side).
                    m_h.append(nc.vector.tensor_scalar(offs_f[:], offs_f[:], rowbase[:, :1],
                                                       float(b * P * C),
                                                       op0=mybir.AluOpType.add,
                                                       op1=mybir.AluOpType.add))
                    sel = s_pool.tile([P, K], I32, tag="offs")
                    m_h.append(nc.vector.tensor_copy(sel[:], offs_f[:]))
                    for k in range(K):
                        g_h.append(nc.gpsimd.indirect_dma_start(
                            out=G[:, k, :], out_offset=None,
                            in_=x_chunks,
                            in_offset=bass.IndirectOffsetOnAxis(ap=sel[:, k:k + 1], axis=0),
                        ))
                if prio:
                    _set_prio(stream_h, -3_000_000)
                    _set_prio(w_h, -3_000_000)
                    _set_prio(m_h, -2_000_000)
                    _set_prio(g_h, -1_000_000)
                if pending is not None:
                    emit_gstage_and_scatter(pending)
                pending = (b, sel, G)
            if pending is not None:
                emit_gstage_and_scatter(pending)
    nc.finalize()
    return nc


def kernel(x: np.ndarray):
    global LAST_EXEC_TIME_NS, LAST_TRACE_DIR, _CACHED_NC
    x = np.ascontiguousarray(np.asarray(x, dtype=np.float32))
    assert x.shape == (N_ROWS, N_COLS), x.shape
    Rs = N_ROWS // N_CORES

    if _CACHED_NC is None:
        _CACHED_NC = _build(Rs, N_COLS)
    nc = _CACHED_NC

    in_maps = [{"x": x[i * Rs:(i + 1) * Rs]} for i in range(N_CORES)]
    tmpdir = None
    if os.environ.get("BASS_TRACE"):
        import tempfile
        tmpdir = tempfile.mkdtemp(prefix="topk_trace_")
        LAST_TRACE_DIR = tmpdir
    res = run_bass_kernel_spmd(nc, in_maps, core_ids=list(range(N_CORES)),
                               tmpdir=tmpdir)
    LAST_EXEC_TIME_NS = res.exec_time_ns

    y = np.concatenate([np.asarray(res.results[i]["y"]).reshape(Rs, N_COLS)
                        for i in range(N_CORES)], axis=0)
    xz = np.concatenate([np.asarray(res.results[i]["xz"]).reshape(Rs, N_COLS)
                         for i in range(N_CORES)], axis=0)
    return y, xz


# revision 43
# speedup vs baseline: 1.0905x; 1.0905x over previous
"""TRN2 Bass kernel: relu + per-row top-32 masking for x [4096, 32768] f32.

kernel(x) -> (relu(x), topk_masked) matching:
    y = relu(x); vals, idx = top_k(y, 32); xz = zeros.at[rows, idx].set(vals)

Sharding: pure data parallel over rows, 8 NeuronCores x [512, 32768].

Per-core algorithm (exact for any input with >=32 positive entries per row):
  stream x in column sub-tiles: relu on ScalarE -> write y; chunk maxes
  (1024 chunks of 32) on VectorE. Top-32 chunks via 4 rounds of DVE
  max8 + match_replace on a copy (selection mask = work != orig, which
  reproduces stable lower-index tie-breaking exactly: every chunk holding
  a top-32 element must rank in the top-32 chunks by (max desc, id asc));
  compact selected chunk ids by max8-extracting (C-c)*sel; indirect-DMA
  gather those 32 chunks/row from DRAM (one [P,1]-offset DMA per chunk
  ordinal - the walrus indirect lowering supports one dynamic offset per
  partition); the same max8+match_replace trick picks the top-32 elements
  of the gathered data; masked chunks (Gf - zapped) are indirect-DMA
  scattered back into the pre-zeroed xz output, so only 1/32 of xz is
  ever written.

Schedule shaping (Tile emits per-engine in-order streams):
  per block: [stream + reduces, M-stage with per-8-group gather issue],
  then the PREVIOUS block's [G-stage + scatters], so the gather critical
  path stays ahead on both DVE and the Pool/SWDGE engine; scatter->scatter
  WAW deps (full-tensor APs, provably disjoint chunks) are stripped;
  priority tiers keep streaming > M-stage > gathers > tail work.
"""

import os
import sys

if "/opt/trn_rl_repo" not in sys.path:
    sys.path.insert(0, "/opt/trn_rl_repo")

import numpy as np

import concourse.bass as bass
import concourse.mybir as mybir
from concourse import bacc
from concourse.bass_utils import run_bass_kernel_spmd
from concourse.tile import TileContext

F32 = mybir.dt.float32
I32 = mybir.dt.int32
I16 = mybir.dt.int16

N_ROWS = 4096
N_COLS = 32768
N_CORES = 8
L = 128          # chunk length
K = 32           # top-k
P = 128          # rows per block (partitions)

LAST_EXEC_TIME_NS = None
LAST_TRACE_DIR = None
_CACHED_NC = None


def _set_prio(handles, delta):
    """Shift bass_priority of emitted instructions (smaller = scheduled first
    among co-pending instructions in Tile's priority heap)."""
    for h in handles:
        ins = getattr(h, "ins", h)
        if ins.bass_priority is not None:
            ins.bass_priority += delta


def _build(R: int, D: int, sub: int = 4096, g_bufs: int = 3, x_bufs: int = 6,
           m_bufs: int = 2, s_bufs: int = 4, prio: bool = True, cl: int = 32,
           mode: str = "ind", scratch: int = 32768):
    assert mode in ("ind", "dg")
    if mode == "dg":
        assert cl == 128, "custom dma_gather path needs 512B chunks + int16 idx"
    C = D // cl
    n_blocks = R // P
    n_sub = D // sub
    sub_chunks = sub // cl

    nc = bacc.Bacc("TRN2", target_bir_lowering=False, debug=False,
                   dynamic_dma_scratch_size=scratch)
    x = nc.declare_dram_parameter("x", [R, D], F32, isOutput=False)
    y = nc.declare_dram_parameter("y", [R, D], F32, isOutput=True)
    xz = nc.declare_dram_parameter("xz", [R, D], F32, isOutput=True)

    x_chunks = x[:].rearrange("r (c l) -> (r c) l", l=cl)
    xz_chunks = xz[:].rearrange("r (c l) -> (r c) l", l=cl)

    with TileContext(nc) as tc:
        with (
            tc.tile_pool(name="consts", bufs=1) as const_pool,
            tc.tile_pool(name="xstream", bufs=x_bufs) as x_pool,
            tc.tile_pool(name="mstage", bufs=m_bufs) as m_pool,
            tc.tile_pool(name="gstage", bufs=g_bufs) as g_pool,
            tc.tile_pool(name="small", bufs=s_bufs) as s_pool,
        ):
            # ids_iota[p, c] = C - c  (max8 extraction then yields ascending chunk id)
            ids_iota_i = const_pool.tile([P, C], I32, tag="ids_iota_i")
            nc.gpsimd.iota(ids_iota_i[:], pattern=[[-1, C]], base=C, channel_multiplier=0)
            ids_iota = const_pool.tile([P, C], F32, tag="ids_iota")
            nc.vector.tensor_copy(ids_iota[:], ids_iota_i[:])
            # rowbase[p, 0] = p*C + C  (global chunk = rowbase - e + block_base)
            rowbase_i = const_pool.tile([P, 1], I32, tag="rowbase_i")
            nc.gpsimd.iota(rowbase_i[:], pattern=[[0, 1]], base=C, channel_multiplier=C)
            rowbase = const_pool.tile([P, 1], F32, tag="rowbase")
            nc.vector.tensor_copy(rowbase[:], rowbase_i[:])

            # Pipelined emission: [stream(b), M(b), gathers(b)] then
            # [gstage(b-1), scatters(b-1)]. Engines execute their scheduled
            # streams in order, so this keeps the gather critical path (reduces
            # -> M-rounds -> offsets -> gather DGE) ahead of the previous
            # block's G-stage on both DVE and Pool.
            pending = None  # (b, offs, G) awaiting gstage + scatters
            # All scatters write the same full-tensor xz AP (the indirect side
            # must have offset 0), so Tile chains them with WAW completion
            # semaphores. The actual chunk destinations are provably disjoint
            # (distinct chunks per row, distinct rows per block), so strip
            # scatter->scatter deps.
            scatter_names = set()

            def emit_gstage_and_scatter(state):
                sb, s_sel, G = state
                Gf = G[:].rearrange("p k l -> p (k l)")
                # top-32 elements of G: zap to 0, then masked = Gf - Gw
                # (kept values are > 0 whenever every row has >= 32 positives,
                #  so zapped zeros are never re-extracted)
                Gw = g_pool.tile([P, K * cl], F32, tag="Gw")
                gx8 = s_pool.tile([P, 8], F32, tag="gx8")
                gsrc = Gf
                for _ in range(K // 8):
                    nc.vector.max(gx8[:], gsrc)
                    nc.vector.match_replace(out=Gw[:], in_to_replace=gx8[:],
                                            in_values=gsrc, imm_value=0.0)
                    gsrc = Gw[:]
                nc.vector.tensor_tensor(out=Gw[:], in0=Gf, in1=Gw[:],
                                        op=mybir.AluOpType.subtract)
                Gw3 = Gw[:].rearrange("p (k l) -> p k l", l=cl)
                if mode == "dg":
                    # scatter-add the masked chunks onto the pre-zeroed xz
                    # (one custom SWDGE instruction; add-to-zero == write)
                    ins = nc.gpsimd.dma_scatter_add(
                        out_ap=xz_chunks[sb * P * C:(sb + 1) * P * C, :],
                        in_ap=Gw3,
                        idxs_ap=s_sel[:],
                        num_idxs=P * K,
                        num_idxs_reg=P * K,
                        elem_size=cl,
                        single_packet=False,
                    )
                    new_ins = [ins]
                else:
                    new_ins = []
                    for k in range(K):
                        new_ins.append(nc.gpsimd.indirect_dma_start(
                            out=xz_chunks,
                            out_offset=bass.IndirectOffsetOnAxis(ap=s_sel[:, k:k + 1], axis=0),
                            in_=Gw3[:, k, :],
                            in_offset=None,
                        ))
                for ins in new_ins:
                    ins = getattr(ins, "ins", ins)
                    for dep in list(ins.sync_dependency_names()):
                        if dep in scatter_names:
                            ins.try_remove_dependency(dep)
                    scatter_names.add(ins.name)

            for b in range(n_blocks):
                r0 = b * P
                M = m_pool.tile([P, C], F32, tag="M")
                stream_h = []
                w_h = []
                for s in range(n_sub):
                    c0 = s * sub
                    xt = x_pool.tile([P, sub], F32, tag="xt")
                    stream_h.append(nc.sync.dma_start(out=xt[:], in_=x[r0:r0 + P, c0:c0 + sub]))
                    stream_h.append(nc.scalar.activation(xt[:], xt[:], mybir.ActivationFunctionType.Relu))
                    w_h.append(nc.sync.dma_start(out=y[r0:r0 + P, c0:c0 + sub], in_=xt[:]))
                    stream_h.append(nc.vector.tensor_reduce(
                        out=M[:, s * sub_chunks:(s + 1) * sub_chunks],
                        in_=xt[:].rearrange("p (c l) -> p c l", l=cl),
                        axis=mybir.AxisListType.X,
                        op=mybir.AluOpType.max,
                    ))

                # top-32 chunks (first round reads M directly, rest in-place on Mw)
                m_h = []
                Mw = m_pool.tile([P, C], F32, tag="Mw")
                mx8 = s_pool.tile([P, 8], F32, tag="mx8")
                src = M
                for _ in range(K // 8):
                    m_h.append(nc.vector.max(mx8[:], src[:]))
                    m_h.append(nc.vector.match_replace(out=Mw[:], in_to_replace=mx8[:],
                                                       in_values=src[:], imm_value=-1.0))
                    src = Mw
                selM = m_pool.tile([P, C], F32, tag="selM")
                m_h.append(nc.vector.tensor_tensor(out=selM[:], in0=Mw[:], in1=M[:],
                                                   op=mybir.AluOpType.not_equal))
                ids = m_pool.tile([P, C], F32, tag="ids")
                m_h.append(nc.vector.tensor_tensor(out=ids[:], in0=selM[:], in1=ids_iota[:],
                                                   op=mybir.AluOpType.mult))
                idsel = s_pool.tile([P, K], F32, tag="idsel")
                offs_f = s_pool.tile([P, K], F32, tag="offs_f")
                g_h = []
                G = g_pool.tile([P, K, cl], F32, tag="G")

                if mode == "ind":
                    # Interleave id-extraction rounds with per-group offset
                    # math and gather issue, so gather DGE overlaps the
                    # remaining DVE id rounds.
                    sel = s_pool.tile([P, K], I32, tag="offs")
                    for r in range(K // 8):
                        sl = slice(r * 8, (r + 1) * 8)
                        m_h.append(nc.vector.max(idsel[:, sl], ids[:]))
                        if r < K // 8 - 1:
                            m_h.append(nc.vector.match_replace(
                                out=ids[:], in_to_replace=idsel[:, sl],
                                in_values=ids[:], imm_value=0.0))
                        m_h.append(nc.vector.tensor_scalar(
                            offs_f[:, sl], idsel[:, sl], -1.0, None,
                            op0=mybir.AluOpType.mult))
                        m_h.append(nc.vector.tensor_scalar(
                            offs_f[:, sl], offs_f[:, sl], rowbase[:, :1],
                            float(b * P * C),
                            op0=mybir.AluOpType.add, op1=mybir.AluOpType.add))
                        m_h.append(nc.vector.tensor_copy(sel[:, sl], offs_f[:, sl]))
                        for k in range(r * 8, (r + 1) * 8):
                            g_h.append(nc.gpsimd.indirect_dma_start(
                                out=G[:, k, :], out_offset=None,
                                in_=x_chunks,
                                in_offset=bass.IndirectOffsetOnAxis(ap=sel[:, k:k + 1], axis=0),
                            ))
                    if prio:
                        _set_prio(stream_h, -3_000_000)
                        _set_prio(w_h, -3_000_000)
                        _set_prio(m_h, -2_000_000)
                        _set_prio(g_h, -1_000_000)
                    if pending is not None:
                        emit_gstage_and_scatter(pending)
                    pending = (b, sel, G)
                    continue

                for r in range(K // 8):
                    m_h.append(nc.vector.max(idsel[:, r * 8:(r + 1) * 8], ids[:]))
                    m_h.append(nc.vector.match_replace(out=ids[:], in_to_replace=idsel[:, r * 8:(r + 1) * 8],
                                                       in_values=ids[:], imm_value=0.0))

                # offsets: block-relative chunk index = rowbase - e
                # = p*C + (C - e) <= P*C - 1, exact in f32.
                m_h.append(nc.vector.tensor_scalar(offs_f[:], idsel[:], -1.0, None,
                                                   op0=mybir.AluOpType.mult))
                if mode == "dg":
                    m_h.append(nc.vector.tensor_scalar(offs_f[:], offs_f[:], rowbase[:, :1],
                                                       None, op0=mybir.AluOpType.add))
                    offs16 = s_pool.tile([P, K], I16, tag="offs16")
                    m_h.append(nc.vector.tensor_copy(offs16[:], offs_f[:]))
                    # dma_gather idx layout: int16 entries wrapped in 16
                    # partitions (entry n at [n % 16, n // 16]), replicated to
                    # all 8 Q7 core groups; gathered segment i lands at
                    # (partition i % 128, slot i // 128). We want G[p, j] =
                    # chunk offs16[p, j], i.e. idx[j*128 + p] = offs16[p, j],
                    # i.e. IDX16[q, j*8 + g] = offs16[g*16 + q, j].
                    sel = s_pool.tile([P, K * 8], I16, tag="idx16")
                    for g in range(8):
                        m_h.append(nc.sync.dma_start(
                            out=sel[0:16, g::8],
                            in_=offs16[g * 16:(g + 1) * 16, :],
                        ))
                    for grp in range(1, 8):
                        m_h.append(nc.sync.dma_start(
                            out=sel[grp * 16:(grp + 1) * 16, :],
                            in_=sel[0:16, :],
                        ))
                    g_h.append(nc.gpsimd.dma_gather(
                        out_ap=G[:],
                        in_ap=x_chunks[b * P * C:(b + 1) * P * C, :],
                        idxs_ap=sel[:],
                        num_idxs=P * K,
                        num_idxs_reg=P * K,
                        elem_size=cl,
                        single_packet=False,
                    ))
                else:
                    # global chunk index = rowbase - e + b*P*C; one indirect
                    # DMA per chunk ordinal (walrus supports one dynamic
                    # offset per partition, 2D SBUF side).
                    m_h.append(nc.vector.tensor_scalar(offs_f[:], offs_f[:], rowbase[:, :1],
                                                       float(b * P * C),
                                                       op0=mybir.AluOpType.add,
                                                       op1=mybir.AluOpType.add))
                    sel = s_pool.tile([P, K], I32, tag="offs")
                    m_h.append(nc.vector.tensor_copy(sel[:], offs_f[:]))
                    for k in range(K):
                        g_h.append(nc.gpsimd.indirect_dma_start(
                            out=G[:, k, :], out_offset=None,
                            in_=x_chunks,
                            in_offset=bass.IndirectOffsetOnAxis(ap=sel[:, k:k + 1], axis=0),
                        ))
                if prio:
                    _set_prio(stream_h, -3_000_000)
                    _set_prio(w_h, -3_000_000)
                    _set_prio(m_h, -2_000_000)
                    _set_prio(g_h, -1_000_000)
                if pending is not None:
                    emit_gstage_and_scatter(pending)
                pending = (b, sel, G)
            if pending is not None:
                emit_gstage_and_scatter(pending)
    nc.finalize()
    return nc


def kernel(x: np.ndarray):
    global LAST_EXEC_TIME_NS, LAST_TRACE_DIR, _CACHED_NC
    x = np.ascontiguousarray(np.asarray(x, dtype=np.float32))
    assert x.shape == (N_ROWS, N_COLS), x.shape
    Rs = N_ROWS // N_CORES

    if _CACHED_NC is None:
        _CACHED_NC = _build(Rs, N_COLS)
    nc = _CACHED_NC

    in_maps = [{"x": x[i * Rs:(i + 1) * Rs]} for i in range(N_CORES)]
    tmpdir = None
    if os.environ.get("BASS_TRACE"):
        import tempfile
        tmpdir = tempfile.mkdtemp(prefix="topk_trace_")
        LAST_TRACE_DIR = tmpdir
    res = run_bass_kernel_spmd(nc, in_maps, core_ids=list(range(N_CORES)),
                               tmpdir=tmpdir)
    LAST_EXEC_TIME_NS = res.exec_time_ns

    y = np.concatenate([np.asarray(res.results[i]["y"]).reshape(Rs, N_COLS)
                        for i in range(N_CORES)], axis=0)
    xz = np.concatenate([np.asarray(res.results[i]["xz"]).reshape(Rs, N_COLS)
                         for i in range(N_CORES)], axis=0)
    return y, xz
